# revision 2
# baseline (speedup 1.0000x reference)
"""Trainium2 Bass kernel for nn_BinaryNN (binary MLP forward pass).

Strategy (8-core data parallel over the batch):
  - Forward of _binarize_weight / _binary_activation is exactly (x > 0), so all
    hidden activations are 0/1 and layers 2-4 are exact integer matmuls -> bf16.
  - concat([x, 1-x]) @ W1b == x @ (W1top - W1bot) + colsum(W1bot): halves K to 784.
    x is split into 3 bf16 chunks (hi+mid+lo, 24 mantissa bits) for fp32-grade
    accuracy on the one real-valued matmul.
  - LayerNorm(scale=1, bias=0) followed by (.>0) reduces to (a > rowmean(a)).
    Row sums arrive as one extra M=1 matmul column (weights augmented with their
    row-sums), broadcast to 128 partitions with a K=1 ones-matmul, and the
    binarization is a single DVE tensor_tensor(is_gt) per tile.
  - Feature-major layout [features, rows] on chip: no transposes anywhere on
    device; the host pre-transposes x and transposes the [10, B] result back.
"""

import sys

if "/opt/trn_rl_repo" not in sys.path:
    sys.path.insert(0, "/opt/trn_rl_repo")

import numpy as np
import ml_dtypes

bf16 = ml_dtypes.bfloat16

N_CORES = 8
B_FULL = 32768
P = 128
RB = 512  # rows per block (PSUM bank = 512 fp32)

D_IN = 784
K1 = 785  # 784 + constant-one row carrying colsum(W1bot)
F1, F2, F3, NC_OUT = 2048, 1024, 512, 10


def _ktiles(n):
    return [(k0, min(P, n - k0)) for k0 in range(0, n, P)]


def build_bass(n_blocks, c1_over_f1):
    import concourse.bass as bass  # noqa: F401
    import concourse.mybir as mybir
    import concourse.tile as tile
    from concourse import bacc

    f32 = mybir.dt.float32
    bf = mybir.dt.bfloat16
    Copy = mybir.ActivationFunctionType.Copy
    is_gt = mybir.AluOpType.is_gt

    R = n_blocks * RB
    nc = bacc.Bacc("TRN2", target_bir_lowering=False, debug=False, num_devices=N_CORES)

    xhi_d = nc.dram_tensor("xhi", [K1, R], bf, kind="ExternalInput")
    xmd_d = nc.dram_tensor("xmd", [K1, R], bf, kind="ExternalInput")
    xlo_d = nc.dram_tensor("xlo", [K1, R], bf, kind="ExternalInput")
    w1_d = nc.dram_tensor("w1m", [K1, F1 + 1], bf, kind="ExternalInput")
    w2_d = nc.dram_tensor("w2m", [F1, F2 + 1], bf, kind="ExternalInput")
    w3_d = nc.dram_tensor("w3m", [F2, F3 + 1], bf, kind="ExternalInput")
    w4_d = nc.dram_tensor("w4m", [F3, NC_OUT], bf, kind="ExternalInput")
    out_d = nc.dram_tensor("out", [NC_OUT, R], f32, kind="ExternalOutput")

    kt1 = _ktiles(K1)  # 7 tiles (6x128 + 17)
    kt2 = _ktiles(F1)  # 16
    kt3 = _ktiles(F2)  # 8
    kt4 = _ktiles(F3)  # 4

    with tile.TileContext(nc) as tc:
        with (
            tc.tile_pool(name="wpool", bufs=1) as wpool,
            tc.tile_pool(name="xpool", bufs=2) as xpool,
            tc.tile_pool(name="bpool", bufs=2) as bpool,
            tc.tile_pool(name="mpool", bufs=3) as mpool,
            tc.tile_pool(name="opool", bufs=2) as opool,
            tc.tile_pool(name="apool", bufs=3, space="PSUM") as apool,
            tc.tile_pool(name="spool", bufs=2, space="PSUM") as spool,
            tc.tile_pool(name="cpool", bufs=2, space="PSUM") as cpool,
        ):
            # ---- persistent weights -------------------------------------
            w1_sb = wpool.tile([P, len(kt1), F1 + 1], bf)
            for k, (k0, ksz) in enumerate(kt1):
                nc.sync.dma_start(out=w1_sb[0:ksz, k, :], in_=w1_d[k0 : k0 + ksz, :])
            w2_sb = wpool.tile([P, len(kt2), F2 + 1], bf)
            for k, (k0, ksz) in enumerate(kt2):
                nc.sync.dma_start(out=w2_sb[0:ksz, k, :], in_=w2_d[k0 : k0 + ksz, :])
            w3_sb = wpool.tile([P, len(kt3), F3 + 1], bf)
            for k, (k0, ksz) in enumerate(kt3):
                nc.sync.dma_start(out=w3_sb[0:ksz, k, :], in_=w3_d[k0 : k0 + ksz, :])
            w4_sb = wpool.tile([P, len(kt4), NC_OUT], bf)
            for k, (k0, ksz) in enumerate(kt4):
                nc.sync.dma_start(out=w4_sb[0:ksz, k, :], in_=w4_d[k0 : k0 + ksz, :])
            ones_sb = wpool.tile([1, P], f32)
            nc.vector.memset(ones_sb[:], 1.0)

            def layer(rhs_list, w_sb, n_mt, sum_col, scale, bias, sink):
                """rhs_list: [(tile, k_idx, ksz)]; sink(m, acc_ap, m_sb_ap)."""
                nmm = len(rhs_list)
                # row-sum column -> per-row mean
                sum_ps = spool.tile([1, RB], f32, tag="sum")
                for i, (t, k, ksz) in enumerate(rhs_list):
                    nc.tensor.matmul(
                        sum_ps[:],
                        w_sb[0:ksz, k, sum_col : sum_col + 1],
                        t[0:ksz, k, :],
                        start=(i == 0),
                        stop=(i == nmm - 1),
                    )
                m_row = mpool.tile([1, RB], f32, tag="m_row")
                nc.scalar.activation(m_row[:], sum_ps[:], Copy, bias=bias, scale=scale)
                m_ps = cpool.tile([P, RB], f32, tag="bcast")
                nc.tensor.matmul(m_ps[:], ones_sb[:], m_row[:], start=True, stop=True)
                m_sb = mpool.tile([P, RB], f32, tag="m_sb")
                nc.scalar.copy(m_sb[:], m_ps[:])
                for m in range(n_mt):
                    acc = apool.tile([P, RB], f32, tag="acc")
                    for i, (t, k, ksz) in enumerate(rhs_list):
                        nc.tensor.matmul(
                            acc[:],
                            w_sb[0:ksz, k, m * P : (m + 1) * P],
                            t[0:ksz, k, :],
                            start=(i == 0),
                            stop=(i == nmm - 1),
                        )
                    sink(m, acc, m_sb)

            for blk in range(n_blocks):
                c0 = blk * RB
                # ---- x loads (3 bf16 splits, feature-major) -------------
                xs = []
                for name, d in (("xhi", xhi_d), ("xmd", xmd_d), ("xlo", xlo_d)):
                    t = xpool.tile([P, len(kt1), RB], bf, tag=name)
                    for k, (k0, ksz) in enumerate(kt1):
                        nc.sync.dma_start(
                            out=t[0:ksz, k, :], in_=d[k0 : k0 + ksz, c0 : c0 + RB]
                        )
                    xs.append(t)

                rhs1 = [(t, k, ksz) for t in xs for k, (k0, ksz) in enumerate(kt1)]

                b1 = bpool.tile([P, len(kt2), RB], bf, tag="b1")

                def sink1(m, acc, m_sb):
                    nc.vector.tensor_tensor(b1[:, m, :], acc[:], m_sb[:], is_gt)

                layer(rhs1, w1_sb, F1 // P, F1, 1.0 / F1, c1_over_f1, sink1)

                rhs2 = [(b1, k, ksz) for k, (k0, ksz) in enumerate(kt2)]
                b2 = bpool.tile([P, len(kt3), RB], bf, tag="b2")

                def sink2(m, acc, m_sb):
                    nc.vector.tensor_tensor(b2[:, m, :], acc[:], m_sb[:], is_gt)

                layer(rhs2, w2_sb, F2 // P, F2, 1.0 / F2, 0.0, sink2)

                rhs3 = [(b2, k, ksz) for k, (k0, ksz) in enumerate(kt3)]
                b3 = bpool.tile([P, len(kt4), RB], bf, tag="b3")

                def sink3(m, acc, m_sb):
                    nc.vector.tensor_tensor(b3[:, m, :], acc[:], m_sb[:], is_gt)

                layer(rhs3, w3_sb, F3 // P, F3, 1.0 / F3, 0.0, sink3)

                # ---- layer 4: plain matmul, no LN -----------------------
                acc4 = apool.tile([NC_OUT, RB], f32, tag="acc")
                for i, (k0, ksz) in enumerate(kt4):
                    nc.tensor.matmul(
                        acc4[:],
                        w4_sb[0:ksz, i, :],
                        b3[0:ksz, i, :],
                        start=(i == 0),
                        stop=(i == len(kt4) - 1),
                    )
                out_sb = opool.tile([NC_OUT, RB], f32, tag="out")
                nc.scalar.copy(out_sb[:], acc4[:])
                nc.sync.dma_start(out=out_d[:, c0 : c0 + RB], in_=out_sb[:])

    nc.compile()
    return nc


def prep_host(x, w1, w2, w3, w4):
    """Returns (per-input dict of full arrays, C1/F1 scalar)."""
    w1b = (w1 > 0).astype(np.float32)
    top, bot = w1b[:D_IN], w1b[D_IN:]
    W1eff = top - bot
    c1 = bot.sum(0)
    W1rows = W1eff.sum(1)
    C1 = float(c1.sum())
    assert np.abs(W1rows).max() <= 256 and c1.max() <= 256
    w1m = np.zeros((K1, F1 + 1), np.float32)
    w1m[:D_IN, :F1] = W1eff
    w1m[D_IN, :F1] = c1
    w1m[:D_IN, F1] = W1rows

    def aug(w):
        wb = (w > 0).astype(np.float32)
        rows = wb.sum(1)
        assert rows.max() <= 256
        return np.concatenate([wb, rows[:, None]], 1)

    w2m, w3m = aug(w2), aug(w3)
    w4m = (w4 > 0).astype(np.float32)

    xT = np.ascontiguousarray(x.T).astype(np.float32)  # [784, B]
    hi = xT.astype(bf16)
    r1 = xT - hi.astype(np.float32)
    md = r1.astype(bf16)
    lo = (r1 - md.astype(np.float32)).astype(bf16)
    B = x.shape[0]
    xhi = np.concatenate([hi, np.ones((1, B), bf16)], 0)
    xmd = np.concatenate([md, np.zeros((1, B), bf16)], 0)
    xlo = np.concatenate([lo, np.zeros((1, B), bf16)], 0)

    arrs = {
        "xhi": xhi,
        "xmd": xmd,
        "xlo": xlo,
        "w1m": w1m.astype(bf16),
        "w2m": w2m.astype(bf16),
        "w3m": w3m.astype(bf16),
        "w4m": w4m.astype(bf16),
    }
    return arrs, C1 / F1


def _fallback_numpy(x, w1, w2, w3, w4, ln1_scale, ln1_bias, ln2_scale, ln2_bias,
                    ln3_scale, ln3_bias):
    """General path (arbitrary LN scale/bias): full fp32 LN on host."""
    h = np.concatenate([x, 1.0 - x], 1).astype(np.float32)
    for w, s, b in ((w1, ln1_scale, ln1_bias), (w2, ln2_scale, ln2_bias),
                    (w3, ln3_scale, ln3_bias)):
        a = h @ (w > 0).astype(np.float32)
        m = a.mean(1, dtype=np.float32, keepdims=True)
        v = np.mean((a - m) ** 2, axis=1, dtype=np.float32, keepdims=True)
        z = (a - m) / np.sqrt(v + 1e-6) * s + b
        h = (z > 0).astype(np.float32)
    return h @ (w4 > 0).astype(np.float32)


_CACHE = {}


def kernel(x, w1, w2, w3, w4, ln1_scale, ln1_bias, ln2_scale, ln2_bias,
           ln3_scale, ln3_bias, _trace=False):
    x = np.asarray(x, np.float32)
    fast = (
        np.all(np.asarray(ln1_scale) == 1) and np.all(np.asarray(ln1_bias) == 0)
        and np.all(np.asarray(ln2_scale) == 1) and np.all(np.asarray(ln2_bias) == 0)
        and np.all(np.asarray(ln3_scale) == 1) and np.all(np.asarray(ln3_bias) == 0)
    )
    if not fast or x.shape[0] % (N_CORES * RB) != 0:
        return _fallback_numpy(
            x, np.asarray(w1), np.asarray(w2), np.asarray(w3), np.asarray(w4),
            np.asarray(ln1_scale), np.asarray(ln1_bias), np.asarray(ln2_scale),
            np.asarray(ln2_bias), np.asarray(ln3_scale), np.asarray(ln3_bias),
        ).astype(np.float32)

    from concourse.bass_utils import run_bass_kernel_spmd

    arrs, c1_over_f1 = prep_host(
        x, np.asarray(w1), np.asarray(w2), np.asarray(w3), np.asarray(w4)
    )
    B = x.shape[0]
    R = B // N_CORES
    n_blocks = R // RB

    key = (n_blocks, round(c1_over_f1, 9))
    if key not in _CACHE:
        _CACHE[key] = build_bass(n_blocks, c1_over_f1)
    nc = _CACHE[key]

    in_maps = []
    for c in range(N_CORES):
        sl = slice(c * R, (c + 1) * R)
        m = {
            "xhi": np.ascontiguousarray(arrs["xhi"][:, sl]),
            "xmd": np.ascontiguousarray(arrs["xmd"][:, sl]),
            "xlo": np.ascontiguousarray(arrs["xlo"][:, sl]),
            "w1m": arrs["w1m"],
            "w2m": arrs["w2m"],
            "w3m": arrs["w3m"],
            "w4m": arrs["w4m"],
        }
        in_maps.append(m)

    res = run_bass_kernel_spmd(
        nc, in_maps, core_ids=list(range(N_CORES)), trace=_trace
    )
    out = np.concatenate([res.results[c]["out"] for c in range(N_CORES)], axis=1)
    if _trace:
        kernel._last_result = res
    return np.ascontiguousarray(out.T).astype(np.float32)


# revision 9
# speedup vs baseline: 1.2570x; 1.2570x over previous
"""Trainium2 Bass kernel for nn_BinaryNN (binary MLP forward pass).

Strategy (8-core data parallel over the batch):
  - Forward of _binarize_weight / _binary_activation is exactly (x > 0), so all
    hidden activations are 0/1 and layers 2-4 are exact integer matmuls -> bf16.
  - concat([x, 1-x]) @ W1b == x @ (W1top - W1bot) + colsum(W1bot): halves K to 784.
    x is split into 3 bf16 chunks (hi+mid+lo, 24 mantissa bits) for fp32-grade
    accuracy on the one real-valued matmul.
  - LayerNorm(scale=1, bias=0) followed by (.>0) reduces to (a > rowmean(a)).
    Row sums arrive as one extra M=1 matmul column (weights augmented with their
    row-sums), broadcast to 128 partitions with a K=1 ones-matmul, and the
    binarization is a single DVE tensor_tensor(is_gt) per tile.
  - Feature-major layout [features, rows] on chip: no transposes anywhere on
    device; the host pre-transposes x and transposes the [10, B] result back.
"""

import sys

if "/opt/trn_rl_repo" not in sys.path:
    sys.path.insert(0, "/opt/trn_rl_repo")

import numpy as np
import ml_dtypes

bf16 = ml_dtypes.bfloat16
fp16 = np.float16
LO_SCALE = 4096.0  # 2**12: keeps the low fp16 chunk of x in the normal range

N_CORES = 8
B_FULL = 32768
P = 128
RB = 512  # rows per block (PSUM bank = 512 fp32)

D_IN = 784
K1 = 785  # 784 + constant-one row carrying colsum(W1bot)
F1, F2, F3, NC_OUT = 2048, 1024, 512, 10


def _ktiles(n):
    return [(k0, min(P, n - k0)) for k0 in range(0, n, P)]


def build_bass(n_blocks, c1_over_f1):
    import concourse.bass as bass  # noqa: F401
    import concourse.mybir as mybir
    import concourse.tile as tile
    from concourse import bacc

    f32 = mybir.dt.float32
    bf = mybir.dt.bfloat16
    f16 = mybir.dt.float16
    Copy = mybir.ActivationFunctionType.Copy
    is_gt = mybir.AluOpType.is_gt

    R = n_blocks * RB
    nc = bacc.Bacc("TRN2", target_bir_lowering=False, debug=False, num_devices=N_CORES)

    xhi_d = nc.dram_tensor("xhi", [K1, R], f16, kind="ExternalInput")
    xlo_d = nc.dram_tensor("xlo", [K1, R], f16, kind="ExternalInput")
    w1h_d = nc.dram_tensor("w1h", [K1, F1 + 1], f16, kind="ExternalInput")
    w1l_d = nc.dram_tensor("w1l", [K1, F1 + 1], f16, kind="ExternalInput")
    w2_d = nc.dram_tensor("w2m", [F1, F2 + 1], bf, kind="ExternalInput")
    w3_d = nc.dram_tensor("w3m", [F2, F3 + 1], bf, kind="ExternalInput")
    w4_d = nc.dram_tensor("w4m", [F3, NC_OUT], bf, kind="ExternalInput")
    out_d = nc.dram_tensor("out", [NC_OUT, R], f32, kind="ExternalOutput")

    kt1 = _ktiles(K1)  # 7 tiles (6x128 + 17)
    kt2 = _ktiles(F1)  # 16
    kt3 = _ktiles(F2)  # 8
    kt4 = _ktiles(F3)  # 4

    with tile.TileContext(nc) as tc:
        with (
            tc.tile_pool(name="wpool", bufs=1) as wpool,
            tc.tile_pool(name="xpool", bufs=2) as xpool,
            tc.tile_pool(name="bpool", bufs=2) as bpool,
            tc.tile_pool(name="mpool", bufs=3) as mpool,
            tc.tile_pool(name="opool", bufs=2) as opool,
            tc.tile_pool(name="apool", bufs=3, space="PSUM") as apool,
            tc.tile_pool(name="spool", bufs=2, space="PSUM") as spool,
            tc.tile_pool(name="cpool", bufs=2, space="PSUM") as cpool,
        ):
            # ---- persistent weights -------------------------------------
            w1h_sb = wpool.tile([P, len(kt1), F1 + 1], f16)
            for k, (k0, ksz) in enumerate(kt1):
                nc.sync.dma_start(out=w1h_sb[0:ksz, k, :], in_=w1h_d[k0 : k0 + ksz, :])
            w1l_sb = wpool.tile([P, len(kt1), F1 + 1], f16)
            for k, (k0, ksz) in enumerate(kt1):
                nc.sync.dma_start(out=w1l_sb[0:ksz, k, :], in_=w1l_d[k0 : k0 + ksz, :])
            w2_sb = wpool.tile([P, len(kt2), F2 + 1], bf)
            for k, (k0, ksz) in enumerate(kt2):
                nc.sync.dma_start(out=w2_sb[0:ksz, k, :], in_=w2_d[k0 : k0 + ksz, :])
            w3_sb = wpool.tile([P, len(kt3), F3 + 1], bf)
            for k, (k0, ksz) in enumerate(kt3):
                nc.sync.dma_start(out=w3_sb[0:ksz, k, :], in_=w3_d[k0 : k0 + ksz, :])
            w4_sb = wpool.tile([P, len(kt4), NC_OUT], bf)
            for k, (k0, ksz) in enumerate(kt4):
                nc.sync.dma_start(out=w4_sb[0:ksz, k, :], in_=w4_d[k0 : k0 + ksz, :])
            ones_sb = wpool.tile([1, P], f32)
            nc.vector.memset(ones_sb[:], 1.0)

            def layer(rhs_list, n_mt, sum_col, scale, bias, sink):
                """rhs_list: [(tile, k_idx, ksz, w_sb)]; sink(m, acc_ap, m_sb_ap)."""
                nmm = len(rhs_list)
                # row-sum column -> per-row mean
                sum_ps = spool.tile([1, RB], f32, tag="sum")
                for i, (t, k, ksz, w) in enumerate(rhs_list):
                    nc.tensor.matmul(
                        sum_ps[:],
                        w[0:ksz, k, sum_col : sum_col + 1],
                        t[0:ksz, k, :],
                        start=(i == 0),
                        stop=(i == nmm - 1),
                    )
                m_row = mpool.tile([1, RB], f32, tag="m_row")
                nc.scalar.activation(m_row[:], sum_ps[:], Copy, bias=bias, scale=scale)
                m_ps = cpool.tile([P, RB], f32, tag="bcast")
                nc.tensor.matmul(m_ps[:], ones_sb[:], m_row[:], start=True, stop=True)
                m_sb = mpool.tile([P, RB], f32, tag="m_sb")
                nc.scalar.copy(m_sb[:], m_ps[:])
                for m in range(n_mt):
                    acc = apool.tile([P, RB], f32, tag="acc")
                    for i, (t, k, ksz, w) in enumerate(rhs_list):
                        nc.tensor.matmul(
                            acc[:],
                            w[0:ksz, k, m * P : (m + 1) * P],
                            t[0:ksz, k, :],
                            start=(i == 0),
                            stop=(i == nmm - 1),
                        )
                    sink(m, acc, m_sb)

            for blk in range(n_blocks):
                c0 = blk * RB
                # ---- x loads (2 fp16 splits, feature-major) -------------
                xs = []
                for name, d in (("xhi", xhi_d), ("xlo", xlo_d)):
                    t = xpool.tile([P, len(kt1), RB], f16, tag=name)
                    for k, (k0, ksz) in enumerate(kt1):
                        nc.sync.dma_start(
                            out=t[0:ksz, k, :], in_=d[k0 : k0 + ksz, c0 : c0 + RB]
                        )
                    xs.append(t)

                rhs1 = [
                    (t, k, ksz, w)
                    for t, w in zip(xs, (w1h_sb, w1l_sb))
                    for k, (k0, ksz) in enumerate(kt1)
                ]

                b1 = bpool.tile([P, len(kt2), RB], bf, tag="b1")

                def sink1(m, acc, m_sb):
                    nc.vector.tensor_tensor(b1[:, m, :], acc[:], m_sb[:], is_gt)

                layer(rhs1, F1 // P, F1, 1.0 / F1, c1_over_f1, sink1)

                rhs2 = [(b1, k, ksz, w2_sb) for k, (k0, ksz) in enumerate(kt2)]
                b2 = bpool.tile([P, len(kt3), RB], bf, tag="b2")

                def sink2(m, acc, m_sb):
                    nc.vector.tensor_tensor(b2[:, m, :], acc[:], m_sb[:], is_gt)

                layer(rhs2, F2 // P, F2, 1.0 / F2, 0.0, sink2)

                rhs3 = [(b2, k, ksz, w3_sb) for k, (k0, ksz) in enumerate(kt3)]
                b3 = bpool.tile([P, len(kt4), RB], bf, tag="b3")

                def sink3(m, acc, m_sb):
                    nc.vector.tensor_tensor(b3[:, m, :], acc[:], m_sb[:], is_gt)

                layer(rhs3, F3 // P, F3, 1.0 / F3, 0.0, sink3)

                # ---- layer 4: plain matmul, no LN -----------------------
                acc4 = apool.tile([NC_OUT, RB], f32, tag="acc")
                for i, (k0, ksz) in enumerate(kt4):
                    nc.tensor.matmul(
                        acc4[:],
                        w4_sb[0:ksz, i, :],
                        b3[0:ksz, i, :],
                        start=(i == 0),
                        stop=(i == len(kt4) - 1),
                    )
                out_sb = opool.tile([NC_OUT, RB], f32, tag="out")
                nc.scalar.copy(out_sb[:], acc4[:])
                nc.sync.dma_start(out=out_d[:, c0 : c0 + RB], in_=out_sb[:])

    nc.compile()
    return nc


def prep_host(x, w1, w2, w3, w4):
    """Returns (per-input dict of full arrays, C1/F1 scalar)."""
    w1b = (w1 > 0).astype(np.float32)
    top, bot = w1b[:D_IN], w1b[D_IN:]
    W1eff = top - bot
    c1 = bot.sum(0)
    W1rows = W1eff.sum(1)
    C1 = float(c1.sum())
    assert np.abs(W1rows).max() <= 256 and c1.max() <= 256
    w1m = np.zeros((K1, F1 + 1), np.float32)
    w1m[:D_IN, :F1] = W1eff
    w1m[D_IN, :F1] = c1
    w1m[:D_IN, F1] = W1rows

    def aug(w):
        wb = (w > 0).astype(np.float32)
        rows = wb.sum(1)
        assert rows.max() <= 256
        return np.concatenate([wb, rows[:, None]], 1)

    w2m, w3m = aug(w2), aug(w3)
    w4m = (w4 > 0).astype(np.float32)

    xT = np.ascontiguousarray(x.T).astype(np.float32)  # [784, B]
    hi = xT.astype(fp16)
    r1 = xT - hi.astype(np.float32)
    lo = (r1 * LO_SCALE).astype(fp16)  # scaled chunk stays fp16-normal
    B = x.shape[0]
    xhi = np.concatenate([hi, np.ones((1, B), fp16)], 0)
    xlo = np.concatenate([lo, np.zeros((1, B), fp16)], 0)

    arrs = {
        "xhi": xhi,
        "xlo": xlo,
        "w1h": w1m.astype(fp16),
        "w1l": (w1m / LO_SCALE).astype(fp16),
        "w2m": w2m.astype(bf16),
        "w3m": w3m.astype(bf16),
        "w4m": w4m.astype(bf16),
    }
    return arrs, C1 / F1


def _fallback_numpy(x, w1, w2, w3, w4, ln1_scale, ln1_bias, ln2_scale, ln2_bias,
                    ln3_scale, ln3_bias):
    """General path (arbitrary LN scale/bias): full fp32 LN on host."""
    h = np.concatenate([x, 1.0 - x], 1).astype(np.float32)
    for w, s, b in ((w1, ln1_scale, ln1_bias), (w2, ln2_scale, ln2_bias),
                    (w3, ln3_scale, ln3_bias)):
        a = h @ (w > 0).astype(np.float32)
        m = a.mean(1, dtype=np.float32, keepdims=True)
        v = np.mean((a - m) ** 2, axis=1, dtype=np.float32, keepdims=True)
        z = (a - m) / np.sqrt(v + 1e-6) * s + b
        h = (z > 0).astype(np.float32)
    return h @ (w4 > 0).astype(np.float32)


_CACHE = {}


def kernel(x, w1, w2, w3, w4, ln1_scale, ln1_bias, ln2_scale, ln2_bias,
           ln3_scale, ln3_bias, _trace=False):
    x = np.asarray(x, np.float32)
    fast = (
        np.all(np.asarray(ln1_scale) == 1) and np.all(np.asarray(ln1_bias) == 0)
        and np.all(np.asarray(ln2_scale) == 1) and np.all(np.asarray(ln2_bias) == 0)
        and np.all(np.asarray(ln3_scale) == 1) and np.all(np.asarray(ln3_bias) == 0)
    )
    if not fast or x.shape[0] % (N_CORES * RB) != 0:
        return _fallback_numpy(
            x, np.asarray(w1), np.asarray(w2), np.asarray(w3), np.asarray(w4),
            np.asarray(ln1_scale), np.asarray(ln1_bias), np.asarray(ln2_scale),
            np.asarray(ln2_bias), np.asarray(ln3_scale), np.asarray(ln3_bias),
        ).astype(np.float32)

    from concourse.bass_utils import run_bass_kernel_spmd

    arrs, c1_over_f1 = prep_host(
        x, np.asarray(w1), np.asarray(w2), np.asarray(w3), np.asarray(w4)
    )
    B = x.shape[0]
    R = B // N_CORES
    n_blocks = R // RB

    key = (n_blocks, round(c1_over_f1, 9))
    if key not in _CACHE:
        _CACHE[key] = build_bass(n_blocks, c1_over_f1)
    nc = _CACHE[key]

    in_maps = []
    for c in range(N_CORES):
        sl = slice(c * R, (c + 1) * R)
        m = {
            "xhi": np.ascontiguousarray(arrs["xhi"][:, sl]),
            "xlo": np.ascontiguousarray(arrs["xlo"][:, sl]),
            "w1h": arrs["w1h"],
            "w1l": arrs["w1l"],
            "w2m": arrs["w2m"],
            "w3m": arrs["w3m"],
            "w4m": arrs["w4m"],
        }
        in_maps.append(m)

    res = run_bass_kernel_spmd(
        nc, in_maps, core_ids=list(range(N_CORES)), trace=_trace
    )
    out = np.concatenate([res.results[c]["out"] for c in range(N_CORES)], axis=1)
    if _trace:
        kernel._last_result = res
    return np.ascontiguousarray(out.T).astype(np.float32)


# revision 17
# speedup vs baseline: 1.5299x; 1.2171x over previous
"""Trainium2 Bass kernel for nn_BinaryNN (binary MLP forward pass).

Strategy (8-core data parallel over the batch):
  - Forward of _binarize_weight / _binary_activation is exactly (x > 0), so all
    hidden activations are 0/1 and layers 2-4 are exact integer matmuls -> bf16.
  - concat([x, 1-x]) @ W1b == x @ (W1top - W1bot) + colsum(W1bot): halves K to 784.
    x is split into 3 bf16 chunks (hi+mid+lo, 24 mantissa bits) for fp32-grade
    accuracy on the one real-valued matmul.
  - LayerNorm(scale=1, bias=0) followed by (.>0) reduces to (a > rowmean(a)).
    Row sums arrive as one extra M=1 matmul column (weights augmented with their
    row-sums), broadcast to 128 partitions with a K=1 ones-matmul, and the
    binarization is a single DVE tensor_tensor(is_gt) per tile.
  - Feature-major layout [features, rows] on chip: no transposes anywhere on
    device; the host pre-transposes x and transposes the [10, B] result back.
"""

import sys

if "/opt/trn_rl_repo" not in sys.path:
    sys.path.insert(0, "/opt/trn_rl_repo")

import numpy as np
import ml_dtypes

bf16 = ml_dtypes.bfloat16
fp16 = np.float16
fp8 = ml_dtypes.float8_e4m3
LO_SCALE = 4096.0  # 2**12: keeps the low fp16 chunk of x in the normal range

# fp8 weight matrices pad their free dim so the DoubleRow "two"-step is 16B-aligned
W2PAD, W3PAD, W4PAD = 1040, 528, 16
NSUM = 3  # row-sum ints (<=48) split into 3 fp8-exact (<=16) columns

N_CORES = 8
B_FULL = 32768
P = 128
RB = 512  # rows per block (PSUM bank = 512 fp32)

D_IN = 784
K1 = 785  # 784 + constant-one row carrying colsum(W1bot)
F1, F2, F3, NC_OUT = 2048, 1024, 512, 10


def _ktiles(n):
    return [(k0, min(P, n - k0)) for k0 in range(0, n, P)]


def build_bass(n_blocks, c1_over_f1):
    import concourse.bass as bass  # noqa: F401
    import concourse.mybir as mybir
    import concourse.tile as tile
    from concourse import bacc

    f32 = mybir.dt.float32
    f16 = mybir.dt.float16
    f8 = mybir.dt.float8e4
    DR = mybir.MatmulPerfMode.DoubleRow
    Copy = mybir.ActivationFunctionType.Copy
    is_gt = mybir.AluOpType.is_gt

    R = n_blocks * RB
    nc = bacc.Bacc("TRN2", target_bir_lowering=False, debug=False, num_devices=N_CORES)

    xhi_d = nc.dram_tensor("xhi", [K1, R], f16, kind="ExternalInput")
    xlo_d = nc.dram_tensor("xlo", [K1, R], f16, kind="ExternalInput")
    w1h_d = nc.dram_tensor("w1h", [K1, F1 + 1], f16, kind="ExternalInput")
    w1l_d = nc.dram_tensor("w1l", [K1, F1 + 1], f16, kind="ExternalInput")
    w2_d = nc.dram_tensor("w2m", [F1, W2PAD], f8, kind="ExternalInput")
    w3_d = nc.dram_tensor("w3m", [F2, W3PAD], f8, kind="ExternalInput")
    w4_d = nc.dram_tensor("w4m", [F3, W4PAD], f8, kind="ExternalInput")
    out_d = nc.dram_tensor("out", [NC_OUT, R], f32, kind="ExternalOutput")

    kt1 = _ktiles(K1)  # 7 tiles (6x128 + 17)
    kt2 = _ktiles(F1)  # 16
    kt3 = _ktiles(F2)  # 8
    kt4 = _ktiles(F3)  # 4

    with tile.TileContext(nc) as tc:
        with (
            tc.tile_pool(name="wpool", bufs=1) as wpool,
            tc.tile_pool(name="xpool", bufs=2) as xpool,
            tc.tile_pool(name="bpool", bufs=2) as bpool,
            tc.tile_pool(name="mpool", bufs=3) as mpool,
            tc.tile_pool(name="opool", bufs=2) as opool,
            tc.tile_pool(name="apool", bufs=3, space="PSUM") as apool,
            tc.tile_pool(name="spool", bufs=2, space="PSUM") as spool,
            tc.tile_pool(name="cpool", bufs=2, space="PSUM") as cpool,
        ):
            # ---- persistent weights -------------------------------------
            w1h_sb = wpool.tile([P, len(kt1), F1 + 1], f16)
            for k, (k0, ksz) in enumerate(kt1):
                nc.sync.dma_start(out=w1h_sb[0:ksz, k, :], in_=w1h_d[k0 : k0 + ksz, :])
            w1l_sb = wpool.tile([P, len(kt1), F1 + 1], f16)
            for k, (k0, ksz) in enumerate(kt1):
                nc.sync.dma_start(out=w1l_sb[0:ksz, k, :], in_=w1l_d[k0 : k0 + ksz, :])
            w2_sb = wpool.tile([P, len(kt2), W2PAD], f8)
            for k, (k0, ksz) in enumerate(kt2):
                nc.sync.dma_start(out=w2_sb[0:ksz, k, :], in_=w2_d[k0 : k0 + ksz, :])
            w3_sb = wpool.tile([P, len(kt3), W3PAD], f8)
            for k, (k0, ksz) in enumerate(kt3):
                nc.sync.dma_start(out=w3_sb[0:ksz, k, :], in_=w3_d[k0 : k0 + ksz, :])
            w4_sb = wpool.tile([P, len(kt4), W4PAD], f8)
            for k, (k0, ksz) in enumerate(kt4):
                nc.sync.dma_start(out=w4_sb[0:ksz, k, :], in_=w4_d[k0 : k0 + ksz, :])
            ones_sb = wpool.tile([NSUM, P], f32)
            nc.vector.memset(ones_sb[:], 1.0)

            def norm_binarize(nw, n_mt, sum_emit, mm_emit, scale, bias, sink):
                """sum_emit(ps): row-sum matmuls -> ps [nw, RB];
                mm_emit(m, acc): main matmuls for m-tile; sink(m, acc, m_sb)."""
                sum_ps = spool.tile([NSUM, RB], f32, tag="sum")
                sum_emit(sum_ps[0:nw, :])
                m_row = mpool.tile([NSUM, RB], f32, tag="m_row")
                nc.scalar.activation(
                    m_row[0:nw, :], sum_ps[0:nw, :], Copy, bias=bias, scale=scale
                )
                m_ps = cpool.tile([P, RB], f32, tag="bcast")
                nc.tensor.matmul(
                    m_ps[:], ones_sb[0:nw, :], m_row[0:nw, :], start=True, stop=True
                )
                m_sb = mpool.tile([P, RB], f32, tag="m_sb")
                nc.scalar.copy(m_sb[:], m_ps[:])
                for m in range(n_mt):
                    acc = apool.tile([P, RB], f32, tag="acc")
                    mm_emit(m, acc)
                    sink(m, acc, m_sb)

            def emit_plain(rhs_list, cols):
                """rhs_list: [(tile, k, ksz, w_sb)]; cols: (c0, width)."""

                def emit(ps):
                    nmm = len(rhs_list)
                    for i, (t, k, ksz, w) in enumerate(rhs_list):
                        nc.tensor.matmul(
                            ps,
                            w[0:ksz, k, cols[0] : cols[0] + cols[1]],
                            t[0:ksz, k, :],
                            start=(i == 0),
                            stop=(i == nmm - 1),
                        )

                return emit

            def emit_dr(b_tile, w_sb, n_kt, cols):
                """DoubleRow fp8: pairs of k-tiles contracted per matmul."""

                def emit(ps):
                    npair = n_kt // 2
                    for i in range(npair):
                        nc.tensor.matmul(
                            ps,
                            w_sb[:, 2 * i : 2 * i + 2, cols[0] : cols[0] + cols[1]],
                            b_tile[:, 2 * i : 2 * i + 2, :],
                            start=(i == 0),
                            stop=(i == npair - 1),
                            perf_mode=DR,
                        )

                return emit

            for blk in range(n_blocks):
                c0 = blk * RB
                # ---- x loads (2 fp16 splits, feature-major) -------------
                xs = []
                for name, d in (("xhi", xhi_d), ("xlo", xlo_d)):
                    t = xpool.tile([P, len(kt1), RB], f16, tag=name)
                    for k, (k0, ksz) in enumerate(kt1):
                        nc.sync.dma_start(
                            out=t[0:ksz, k, :], in_=d[k0 : k0 + ksz, c0 : c0 + RB]
                        )
                    xs.append(t)

                rhs1 = [
                    (t, k, ksz, w)
                    for t, w in zip(xs, (w1h_sb, w1l_sb))
                    for k, (k0, ksz) in enumerate(kt1)
                ]

                b1 = bpool.tile([P, len(kt2), RB], f8, tag="b1")

                def sink1(m, acc, m_sb):
                    nc.vector.tensor_tensor(b1[:, m, :], acc[:], m_sb[:], is_gt)

                norm_binarize(
                    1,
                    F1 // P,
                    emit_plain(rhs1, (F1, 1)),
                    lambda m, acc: emit_plain(rhs1, (m * P, P))(acc[:]),
                    1.0 / F1,
                    c1_over_f1,
                    sink1,
                )

                b2 = bpool.tile([P, len(kt3), RB], f8, tag="b2")

                def sink2(m, acc, m_sb):
                    nc.vector.tensor_tensor(b2[:, m, :], acc[:], m_sb[:], is_gt)

                norm_binarize(
                    NSUM,
                    F2 // P,
                    emit_dr(b1, w2_sb, len(kt2), (F2, NSUM)),
                    lambda m, acc: emit_dr(b1, w2_sb, len(kt2), (m * P, P))(acc[:]),
                    1.0 / F2,
                    0.0,
                    sink2,
                )

                b3 = bpool.tile([P, len(kt4), RB], f8, tag="b3")

                def sink3(m, acc, m_sb):
                    nc.vector.tensor_tensor(b3[:, m, :], acc[:], m_sb[:], is_gt)

                norm_binarize(
                    NSUM,
                    F3 // P,
                    emit_dr(b2, w3_sb, len(kt3), (F3, NSUM)),
                    lambda m, acc: emit_dr(b2, w3_sb, len(kt3), (m * P, P))(acc[:]),
                    1.0 / F3,
                    0.0,
                    sink3,
                )

                # ---- layer 4: plain DoubleRow matmul, no LN -------------
                acc4 = apool.tile([NC_OUT, RB], f32, tag="acc")
                emit_dr(b3, w4_sb, len(kt4), (0, NC_OUT))(acc4[:])
                out_sb = opool.tile([NC_OUT, RB], f32, tag="out")
                nc.scalar.copy(out_sb[:], acc4[:])
                nc.sync.dma_start(out=out_d[:, c0 : c0 + RB], in_=out_sb[:])

    nc.compile()
    return nc


def prep_host(x, w1, w2, w3, w4):
    """Returns (per-input dict of full arrays, C1/F1 scalar)."""
    w1b = (w1 > 0).astype(np.float32)
    top, bot = w1b[:D_IN], w1b[D_IN:]
    W1eff = top - bot
    c1 = bot.sum(0)
    W1rows = W1eff.sum(1)
    C1 = float(c1.sum())
    assert np.abs(W1rows).max() <= 256 and c1.max() <= 256
    w1m = np.zeros((K1, F1 + 1), np.float32)
    w1m[:D_IN, :F1] = W1eff
    w1m[D_IN, :F1] = c1
    w1m[:D_IN, F1] = W1rows

    def aug8(w, width):
        """fp8 layout: [binary cols | 3-way split of row-sums | zero pad]."""
        wb = (w > 0).astype(np.float32)
        nf = wb.shape[1]
        rows = wb.sum(1)
        assert rows.max() <= 3 * 16, rows.max()
        out = np.zeros((wb.shape[0], width), np.float32)
        out[:, :nf] = wb
        rem = rows
        for i in range(NSUM):
            c = np.minimum(rem, 16.0)
            out[:, nf + i] = c
            rem = rem - c
        return out.astype(fp8)

    w2m, w3m = aug8(w2, W2PAD), aug8(w3, W3PAD)
    w4m = np.zeros((F3, W4PAD), np.float32)
    w4m[:, :NC_OUT] = (w4 > 0).astype(np.float32)
    w4m = w4m.astype(fp8)

    xT = np.ascontiguousarray(x.T).astype(np.float32)  # [784, B]
    hi = xT.astype(fp16)
    r1 = xT - hi.astype(np.float32)
    lo = (r1 * LO_SCALE).astype(fp16)  # scaled chunk stays fp16-normal
    B = x.shape[0]
    xhi = np.concatenate([hi, np.ones((1, B), fp16)], 0)
    xlo = np.concatenate([lo, np.zeros((1, B), fp16)], 0)

    arrs = {
        "xhi": xhi,
        "xlo": xlo,
        "w1h": w1m.astype(fp16),
        "w1l": (w1m / LO_SCALE).astype(fp16),
        "w2m": w2m,
        "w3m": w3m,
        "w4m": w4m,
    }
    return arrs, C1 / F1


def _fallback_numpy(x, w1, w2, w3, w4, ln1_scale, ln1_bias, ln2_scale, ln2_bias,
                    ln3_scale, ln3_bias):
    """General path (arbitrary LN scale/bias): full fp32 LN on host."""
    h = np.concatenate([x, 1.0 - x], 1).astype(np.float32)
    for w, s, b in ((w1, ln1_scale, ln1_bias), (w2, ln2_scale, ln2_bias),
                    (w3, ln3_scale, ln3_bias)):
        a = h @ (w > 0).astype(np.float32)
        m = a.mean(1, dtype=np.float32, keepdims=True)
        v = np.mean((a - m) ** 2, axis=1, dtype=np.float32, keepdims=True)
        z = (a - m) / np.sqrt(v + 1e-6) * s + b
        h = (z > 0).astype(np.float32)
    return h @ (w4 > 0).astype(np.float32)


_CACHE = {}


def kernel(x, w1, w2, w3, w4, ln1_scale, ln1_bias, ln2_scale, ln2_bias,
           ln3_scale, ln3_bias, _trace=False):
    x = np.asarray(x, np.float32)
    fast = (
        np.all(np.asarray(ln1_scale) == 1) and np.all(np.asarray(ln1_bias) == 0)
        and np.all(np.asarray(ln2_scale) == 1) and np.all(np.asarray(ln2_bias) == 0)
        and np.all(np.asarray(ln3_scale) == 1) and np.all(np.asarray(ln3_bias) == 0)
    )
    if not fast or x.shape[0] % (N_CORES * RB) != 0:
        return _fallback_numpy(
            x, np.asarray(w1), np.asarray(w2), np.asarray(w3), np.asarray(w4),
            np.asarray(ln1_scale), np.asarray(ln1_bias), np.asarray(ln2_scale),
            np.asarray(ln2_bias), np.asarray(ln3_scale), np.asarray(ln3_bias),
        ).astype(np.float32)

    from concourse.bass_utils import run_bass_kernel_spmd

    arrs, c1_over_f1 = prep_host(
        x, np.asarray(w1), np.asarray(w2), np.asarray(w3), np.asarray(w4)
    )
    B = x.shape[0]
    R = B // N_CORES
    n_blocks = R // RB

    key = (n_blocks, round(c1_over_f1, 9))
    if key not in _CACHE:
        _CACHE[key] = build_bass(n_blocks, c1_over_f1)
    nc = _CACHE[key]

    in_maps = []
    for c in range(N_CORES):
        sl = slice(c * R, (c + 1) * R)
        m = {
            "xhi": np.ascontiguousarray(arrs["xhi"][:, sl]),
            "xlo": np.ascontiguousarray(arrs["xlo"][:, sl]),
            "w1h": arrs["w1h"],
            "w1l": arrs["w1l"],
            "w2m": arrs["w2m"],
            "w3m": arrs["w3m"],
            "w4m": arrs["w4m"],
        }
        in_maps.append(m)

    res = run_bass_kernel_spmd(
        nc, in_maps, core_ids=list(range(N_CORES)), trace=_trace
    )
    out = np.concatenate([res.results[c]["out"] for c in range(N_CORES)], axis=1)
    if _trace:
        kernel._last_result = res
    return np.ascontiguousarray(out.T).astype(np.float32)


# revision 23
# speedup vs baseline: 1.6537x; 1.0809x over previous
"""Trainium2 Bass kernel for nn_BinaryNN (binary MLP forward pass).

Strategy (8-core data parallel over the batch):
  - Forward of _binarize_weight / _binary_activation is exactly (x > 0), so all
    hidden activations are 0/1 and layers 2-4 are exact integer matmuls -> bf16.
  - concat([x, 1-x]) @ W1b == x @ (W1top - W1bot) + colsum(W1bot): halves K to 784.
    x is split into 3 bf16 chunks (hi+mid+lo, 24 mantissa bits) for fp32-grade
    accuracy on the one real-valued matmul.
  - LayerNorm(scale=1, bias=0) followed by (.>0) reduces to (a > rowmean(a)).
    Row sums arrive as one extra M=1 matmul column (weights augmented with their
    row-sums), broadcast to 128 partitions with a K=1 ones-matmul, and the
    binarization is a single DVE tensor_tensor(is_gt) per tile.
  - Feature-major layout [features, rows] on chip: no transposes anywhere on
    device; the host pre-transposes x and transposes the [10, B] result back.
"""

import sys

if "/opt/trn_rl_repo" not in sys.path:
    sys.path.insert(0, "/opt/trn_rl_repo")

import numpy as np
import ml_dtypes

bf16 = ml_dtypes.bfloat16
fp16 = np.float16
fp8 = ml_dtypes.float8_e4m3
LO_SCALE = 4096.0  # 2**12: keeps the low fp16 chunk of x in the normal range

# fp8 weight matrices pad their free dim so the DoubleRow "two"-step is 16B-aligned
W2PAD, W3PAD, W4PAD = 1040, 528, 16
NSUM = 3  # row-sum ints (<=48) split into 3 fp8-exact (<=16) columns

N_CORES = 8
B_FULL = 32768
P = 128
RB = 512  # rows per block (PSUM bank = 512 fp32)

D_IN = 784
K1 = 785  # 784 + constant-one row carrying colsum(W1bot)
KC = K1 + D_IN  # 1569: hi chunk (with ones row) and scaled lo chunk stacked in K
F1, F2, F3, NC_OUT = 2048, 1024, 512, 10


def _ktiles(n):
    return [(k0, min(P, n - k0)) for k0 in range(0, n, P)]


def build_bass(n_blocks, c1_over_f1):
    import concourse.bass as bass  # noqa: F401
    import concourse.mybir as mybir
    import concourse.tile as tile
    from concourse import bacc

    f32 = mybir.dt.float32
    f16 = mybir.dt.float16
    f8 = mybir.dt.float8e4
    DR = mybir.MatmulPerfMode.DoubleRow
    Copy = mybir.ActivationFunctionType.Copy
    is_gt = mybir.AluOpType.is_gt

    R = n_blocks * RB
    nc = bacc.Bacc("TRN2", target_bir_lowering=False, debug=False, num_devices=N_CORES)

    xc_d = nc.dram_tensor("xc", [KC, R], f16, kind="ExternalInput")
    w1_d = nc.dram_tensor("w1c", [KC, F1 + 1], f16, kind="ExternalInput")
    w2_d = nc.dram_tensor("w2m", [F1, W2PAD], f8, kind="ExternalInput")
    w3_d = nc.dram_tensor("w3m", [F2, W3PAD], f8, kind="ExternalInput")
    w4_d = nc.dram_tensor("w4m", [F3, W4PAD], f8, kind="ExternalInput")
    out_d = nc.dram_tensor("out", [NC_OUT, R], f32, kind="ExternalOutput")

    kt1 = _ktiles(KC)  # 13 tiles (12x128 + 33)
    kt2 = _ktiles(F1)  # 16
    kt3 = _ktiles(F2)  # 8
    kt4 = _ktiles(F3)  # 4

    with tile.TileContext(nc) as tc:
        with (
            tc.tile_pool(name="wpool", bufs=1) as wpool,
            tc.tile_pool(name="xpool", bufs=2) as xpool,
            tc.tile_pool(name="bpool", bufs=2) as bpool,
            tc.tile_pool(name="mpool", bufs=3) as mpool,
            tc.tile_pool(name="opool", bufs=2) as opool,
            tc.tile_pool(name="apool", bufs=3, space="PSUM") as apool,
            tc.tile_pool(name="spool", bufs=2, space="PSUM") as spool,
            tc.tile_pool(name="cpool", bufs=2, space="PSUM") as cpool,
        ):
            # ---- persistent weights -------------------------------------
            # w1 loads column-chunked so early m-tiles can start before the
            # whole 6.4MB matrix lands.
            w1_sb = wpool.tile([P, len(kt1), F1 + 1], f16)
            for c0w in range(0, F1 + 1, 512):
                cw = min(512, F1 + 1 - c0w)
                for k, (k0, ksz) in enumerate(kt1):
                    nc.sync.dma_start(
                        out=w1_sb[0:ksz, k, c0w : c0w + cw],
                        in_=w1_d[k0 : k0 + ksz, c0w : c0w + cw],
                    )

            # block-0 x prefetch ahead of the later-needed w2/w3/w4
            x_tiles = {}

            def load_x(blk):
                t = xpool.tile([P, len(kt1), RB], f16, tag="xc")
                c0 = blk * RB
                for k, (k0, ksz) in enumerate(kt1):
                    nc.sync.dma_start(
                        out=t[0:ksz, k, :], in_=xc_d[k0 : k0 + ksz, c0 : c0 + RB]
                    )
                x_tiles[blk] = t

            load_x(0)

            w2_sb = wpool.tile([P, len(kt2), W2PAD], f8)
            for k, (k0, ksz) in enumerate(kt2):
                nc.sync.dma_start(out=w2_sb[0:ksz, k, :], in_=w2_d[k0 : k0 + ksz, :])
            w3_sb = wpool.tile([P, len(kt3), W3PAD], f8)
            for k, (k0, ksz) in enumerate(kt3):
                nc.sync.dma_start(out=w3_sb[0:ksz, k, :], in_=w3_d[k0 : k0 + ksz, :])
            w4_sb = wpool.tile([P, len(kt4), W4PAD], f8)
            for k, (k0, ksz) in enumerate(kt4):
                nc.sync.dma_start(out=w4_sb[0:ksz, k, :], in_=w4_d[k0 : k0 + ksz, :])
            ones_sb = wpool.tile([NSUM, P], f32)
            nc.vector.memset(ones_sb[:], 1.0)

            def norm_binarize(nw, n_mt, sum_emit, mm_emit, scale, bias, sink):
                """sum_emit(ps): row-sum matmuls -> ps [nw, RB];
                mm_emit(m, acc): main matmuls for m-tile; sink(m, acc, m_sb)."""
                sum_ps = spool.tile([NSUM, RB], f32, tag="sum")
                sum_emit(sum_ps[0:nw, :])
                m_row = mpool.tile([NSUM, RB], f32, tag="m_row")
                nc.scalar.activation(
                    m_row[0:nw, :], sum_ps[0:nw, :], Copy, bias=bias, scale=scale
                )
                m_ps = cpool.tile([P, RB], f32, tag="bcast")
                nc.tensor.matmul(
                    m_ps[:], ones_sb[0:nw, :], m_row[0:nw, :], start=True, stop=True
                )
                m_sb = mpool.tile([P, RB], f32, tag="m_sb")
                nc.scalar.copy(m_sb[:], m_ps[:])
                for m in range(n_mt):
                    acc = apool.tile([P, RB], f32, tag="acc")
                    mm_emit(m, acc)
                    sink(m, acc, m_sb)

            def emit_plain(rhs_list, cols):
                """rhs_list: [(tile, k, ksz, w_sb)]; cols: (c0, width)."""

                def emit(ps):
                    nmm = len(rhs_list)
                    for i, (t, k, ksz, w) in enumerate(rhs_list):
                        nc.tensor.matmul(
                            ps,
                            w[0:ksz, k, cols[0] : cols[0] + cols[1]],
                            t[0:ksz, k, :],
                            start=(i == 0),
                            stop=(i == nmm - 1),
                        )

                return emit

            def emit_dr(b_tile, w_sb, n_kt, cols):
                """DoubleRow fp8: pairs of k-tiles contracted per matmul."""

                def emit(ps):
                    npair = n_kt // 2
                    for i in range(npair):
                        nc.tensor.matmul(
                            ps,
                            w_sb[:, 2 * i : 2 * i + 2, cols[0] : cols[0] + cols[1]],
                            b_tile[:, 2 * i : 2 * i + 2, :],
                            start=(i == 0),
                            stop=(i == npair - 1),
                            perf_mode=DR,
                        )

                return emit

            for blk in range(n_blocks):
                c0 = blk * RB
                if blk not in x_tiles:
                    load_x(blk)
                xt = x_tiles.pop(blk)
                if blk + 1 < n_blocks:
                    load_x(blk + 1)  # prefetch next block's x

                rhs1 = [(xt, k, ksz, w1_sb) for k, (k0, ksz) in enumerate(kt1)]

                b1 = bpool.tile([P, len(kt2), RB], f8, tag="b1")

                def sink1(m, acc, m_sb):
                    nc.vector.tensor_tensor(b1[:, m, :], acc[:], m_sb[:], is_gt)

                norm_binarize(
                    1,
                    F1 // P,
                    emit_plain(rhs1, (F1, 1)),
                    lambda m, acc: emit_plain(rhs1, (m * P, P))(acc[:]),
                    1.0 / F1,
                    c1_over_f1,
                    sink1,
                )

                b2 = bpool.tile([P, len(kt3), RB], f8, tag="b2")

                def sink2(m, acc, m_sb):
                    nc.vector.tensor_tensor(b2[:, m, :], acc[:], m_sb[:], is_gt)

                norm_binarize(
                    NSUM,
                    F2 // P,
                    emit_dr(b1, w2_sb, len(kt2), (F2, NSUM)),
                    lambda m, acc: emit_dr(b1, w2_sb, len(kt2), (m * P, P))(acc[:]),
                    1.0 / F2,
                    0.0,
                    sink2,
                )

                b3 = bpool.tile([P, len(kt4), RB], f8, tag="b3")

                def sink3(m, acc, m_sb):
                    nc.vector.tensor_tensor(b3[:, m, :], acc[:], m_sb[:], is_gt)

                norm_binarize(
                    NSUM,
                    F3 // P,
                    emit_dr(b2, w3_sb, len(kt3), (F3, NSUM)),
                    lambda m, acc: emit_dr(b2, w3_sb, len(kt3), (m * P, P))(acc[:]),
                    1.0 / F3,
                    0.0,
                    sink3,
                )

                # ---- layer 4: plain DoubleRow matmul, no LN -------------
                acc4 = apool.tile([NC_OUT, RB], f32, tag="acc")
                emit_dr(b3, w4_sb, len(kt4), (0, NC_OUT))(acc4[:])
                out_sb = opool.tile([NC_OUT, RB], f32, tag="out")
                nc.scalar.copy(out_sb[:], acc4[:])
                nc.sync.dma_start(out=out_d[:, c0 : c0 + RB], in_=out_sb[:])

    nc.compile()
    return nc


def prep_host(x, w1, w2, w3, w4):
    """Returns (per-input dict of full arrays, C1/F1 scalar)."""
    w1b = (w1 > 0).astype(np.float32)
    top, bot = w1b[:D_IN], w1b[D_IN:]
    W1eff = top - bot
    c1 = bot.sum(0)
    W1rows = W1eff.sum(1)
    C1 = float(c1.sum())
    assert np.abs(W1rows).max() <= 256 and c1.max() <= 256
    w1m = np.zeros((K1, F1 + 1), np.float32)
    w1m[:D_IN, :F1] = W1eff
    w1m[D_IN, :F1] = c1
    w1m[:D_IN, F1] = W1rows

    def aug8(w, width):
        """fp8 layout: [binary cols | 3-way split of row-sums | zero pad]."""
        wb = (w > 0).astype(np.float32)
        nf = wb.shape[1]
        rows = wb.sum(1)
        assert rows.max() <= 3 * 16, rows.max()
        out = np.zeros((wb.shape[0], width), np.float32)
        out[:, :nf] = wb
        rem = rows
        for i in range(NSUM):
            c = np.minimum(rem, 16.0)
            out[:, nf + i] = c
            rem = rem - c
        return out.astype(fp8)

    w2m, w3m = aug8(w2, W2PAD), aug8(w3, W3PAD)
    w4m = np.zeros((F3, W4PAD), np.float32)
    w4m[:, :NC_OUT] = (w4 > 0).astype(np.float32)
    w4m = w4m.astype(fp8)

    xT = np.ascontiguousarray(x.T).astype(np.float32)  # [784, B]
    hi = xT.astype(fp16)
    r1 = xT - hi.astype(np.float32)
    lo = (r1 * LO_SCALE).astype(fp16)  # scaled chunk stays fp16-normal
    B = x.shape[0]
    # single K-stacked operand: [hi; ones; lo] against [w1m; w1m/LO_SCALE]
    xc = np.concatenate([hi, np.ones((1, B), fp16), lo], 0)  # [1569, B]
    w1c = np.concatenate(
        [w1m.astype(fp16), (w1m[:D_IN] / LO_SCALE).astype(fp16)], 0
    )  # [1569, 2049]

    arrs = {
        "xc": xc,
        "w1c": w1c,
        "w2m": w2m,
        "w3m": w3m,
        "w4m": w4m,
    }
    return arrs, C1 / F1


def _fallback_numpy(x, w1, w2, w3, w4, ln1_scale, ln1_bias, ln2_scale, ln2_bias,
                    ln3_scale, ln3_bias):
    """General path (arbitrary LN scale/bias): full fp32 LN on host."""
    h = np.concatenate([x, 1.0 - x], 1).astype(np.float32)
    for w, s, b in ((w1, ln1_scale, ln1_bias), (w2, ln2_scale, ln2_bias),
                    (w3, ln3_scale, ln3_bias)):
        a = h @ (w > 0).astype(np.float32)
        m = a.mean(1, dtype=np.float32, keepdims=True)
        v = np.mean((a - m) ** 2, axis=1, dtype=np.float32, keepdims=True)
        z = (a - m) / np.sqrt(v + 1e-6) * s + b
        h = (z > 0).astype(np.float32)
    return h @ (w4 > 0).astype(np.float32)


_CACHE = {}


def kernel(x, w1, w2, w3, w4, ln1_scale, ln1_bias, ln2_scale, ln2_bias,
           ln3_scale, ln3_bias, _trace=False):
    x = np.asarray(x, np.float32)
    fast = (
        np.all(np.asarray(ln1_scale) == 1) and np.all(np.asarray(ln1_bias) == 0)
        and np.all(np.asarray(ln2_scale) == 1) and np.all(np.asarray(ln2_bias) == 0)
        and np.all(np.asarray(ln3_scale) == 1) and np.all(np.asarray(ln3_bias) == 0)
    )
    if not fast or x.shape[0] % (N_CORES * RB) != 0:
        return _fallback_numpy(
            x, np.asarray(w1), np.asarray(w2), np.asarray(w3), np.asarray(w4),
            np.asarray(ln1_scale), np.asarray(ln1_bias), np.asarray(ln2_scale),
            np.asarray(ln2_bias), np.asarray(ln3_scale), np.asarray(ln3_bias),
        ).astype(np.float32)

    from concourse.bass_utils import run_bass_kernel_spmd

    arrs, c1_over_f1 = prep_host(
        x, np.asarray(w1), np.asarray(w2), np.asarray(w3), np.asarray(w4)
    )
    B = x.shape[0]
    R = B // N_CORES
    n_blocks = R // RB

    key = (n_blocks, round(c1_over_f1, 9))
    if key not in _CACHE:
        _CACHE[key] = build_bass(n_blocks, c1_over_f1)
    nc = _CACHE[key]

    in_maps = []
    for c in range(N_CORES):
        sl = slice(c * R, (c + 1) * R)
        m = {
            "xc": np.ascontiguousarray(arrs["xc"][:, sl]),
            "w1c": arrs["w1c"],
            "w2m": arrs["w2m"],
            "w3m": arrs["w3m"],
            "w4m": arrs["w4m"],
        }
        in_maps.append(m)

    res = run_bass_kernel_spmd(
        nc, in_maps, core_ids=list(range(N_CORES)), trace=_trace
    )
    out = np.concatenate([res.results[c]["out"] for c in range(N_CORES)], axis=1)
    if _trace:
        kernel._last_result = res
    return np.ascontiguousarray(out.T).astype(np.float32)


# revision 27
# speedup vs baseline: 1.7865x; 1.0803x over previous
"""Trainium2 Bass kernel for nn_BinaryNN (binary MLP forward pass).

Strategy (8-core data parallel over the batch):
  - Forward of _binarize_weight / _binary_activation is exactly (x > 0), so all
    hidden activations are 0/1 and layers 2-4 are exact integer matmuls -> bf16.
  - concat([x, 1-x]) @ W1b == x @ (W1top - W1bot) + colsum(W1bot): halves K to 784.
    x is split into 3 bf16 chunks (hi+mid+lo, 24 mantissa bits) for fp32-grade
    accuracy on the one real-valued matmul.
  - LayerNorm(scale=1, bias=0) followed by (.>0) reduces to (a > rowmean(a)).
    Row sums arrive as one extra M=1 matmul column (weights augmented with their
    row-sums), broadcast to 128 partitions with a K=1 ones-matmul, and the
    binarization is a single DVE tensor_tensor(is_gt) per tile.
  - Feature-major layout [features, rows] on chip: no transposes anywhere on
    device; the host pre-transposes x and transposes the [10, B] result back.
"""

import sys

if "/opt/trn_rl_repo" not in sys.path:
    sys.path.insert(0, "/opt/trn_rl_repo")

import numpy as np
import ml_dtypes

bf16 = ml_dtypes.bfloat16
fp16 = np.float16
fp8 = ml_dtypes.float8_e4m3
LO_SCALE = 4096.0  # 2**12: keeps the low fp16 chunk of x in the normal range

# fp8 weight matrices pad their free dim so the DoubleRow "two"-step is 16B-aligned
W2PAD, W3PAD, W4PAD = 1040, 528, 16
NSUM = 3  # row-sum ints (<=48) split into 3 fp8-exact (<=16) columns

N_CORES = 8
B_FULL = 32768
P = 128
RB = 512  # rows per block (PSUM bank = 512 fp32)

D_IN = 784
K1 = 785  # 784 + constant-one row carrying colsum(W1bot)
KC = K1 + D_IN  # 1569: hi chunk (with ones row) and scaled lo chunk stacked in K
KP = 1664  # KC zero-padded to 13*128 so x/w1 move as single 3D-AP DMAs
F1, F2, F3, NC_OUT = 2048, 1024, 512, 10


def _ktiles(n):
    return [(k0, min(P, n - k0)) for k0 in range(0, n, P)]


def build_bass(n_blocks, c1_over_f1):
    import concourse.bass as bass  # noqa: F401
    import concourse.mybir as mybir
    import concourse.tile as tile
    from concourse import bacc

    f32 = mybir.dt.float32
    f16 = mybir.dt.float16
    f8 = mybir.dt.float8e4
    DR = mybir.MatmulPerfMode.DoubleRow
    Copy = mybir.ActivationFunctionType.Copy
    is_gt = mybir.AluOpType.is_gt

    R = n_blocks * RB
    nc = bacc.Bacc("TRN2", target_bir_lowering=False, debug=False, num_devices=N_CORES)

    xc_d = nc.dram_tensor("xc", [KP, R], f16, kind="ExternalInput")
    w1_d = nc.dram_tensor("w1c", [KP, F1 + 1], f16, kind="ExternalInput")
    w2_d = nc.dram_tensor("w2m", [F1, W2PAD], f8, kind="ExternalInput")
    w3_d = nc.dram_tensor("w3m", [F2, W3PAD], f8, kind="ExternalInput")
    w4_d = nc.dram_tensor("w4m", [F3, W4PAD], f8, kind="ExternalInput")
    out_d = nc.dram_tensor("out", [NC_OUT, R], f32, kind="ExternalOutput")

    kt1 = _ktiles(KP)  # 13 tiles of 128
    kt2 = _ktiles(F1)  # 16
    kt3 = _ktiles(F2)  # 8
    kt4 = _ktiles(F3)  # 4

    with tile.TileContext(nc) as tc:
        with (
            tc.tile_pool(name="wpool", bufs=1) as wpool,
            tc.tile_pool(name="xpool", bufs=2) as xpool,
            tc.tile_pool(name="bpool", bufs=2) as bpool,
            tc.tile_pool(name="mpool", bufs=3) as mpool,
            tc.tile_pool(name="opool", bufs=2) as opool,
            tc.tile_pool(name="apool", bufs=3, space="PSUM") as apool,
            tc.tile_pool(name="spool", bufs=2, space="PSUM") as spool,
            tc.tile_pool(name="cpool", bufs=2, space="PSUM") as cpool,
        ):
            # ---- persistent weights (single 3D-AP DMAs) -----------------
            wr1 = w1_d[:, :].rearrange("(t p) j -> p t j", p=P)
            w1_sb = wpool.tile([P, len(kt1), F1 + 1], f16)
            # column-chunked so early m-tiles start before all of w1 lands
            for c0w in range(0, F1 + 1, 512):
                cw = min(512, F1 + 1 - c0w)
                nc.sync.dma_start(
                    out=w1_sb[:, :, c0w : c0w + cw], in_=wr1[:, :, c0w : c0w + cw]
                )

            # block-0 x prefetch ahead of the later-needed w2/w3/w4
            xr = xc_d[:, :].rearrange("(t p) r -> p t r", p=P)
            x_tiles = {}

            def load_x(blk):
                t = xpool.tile([P, len(kt1), RB], f16, tag="xc")
                c0 = blk * RB
                nc.sync.dma_start(out=t[:], in_=xr[:, :, c0 : c0 + RB])
                x_tiles[blk] = t

            load_x(0)

            w2_sb = wpool.tile([P, len(kt2), W2PAD], f8)
            nc.sync.dma_start(
                out=w2_sb[:], in_=w2_d[:, :].rearrange("(t p) j -> p t j", p=P)
            )
            w3_sb = wpool.tile([P, len(kt3), W3PAD], f8)
            nc.sync.dma_start(
                out=w3_sb[:], in_=w3_d[:, :].rearrange("(t p) j -> p t j", p=P)
            )
            w4_sb = wpool.tile([P, len(kt4), W4PAD], f8)
            nc.sync.dma_start(
                out=w4_sb[:], in_=w4_d[:, :].rearrange("(t p) j -> p t j", p=P)
            )
            ones_sb = wpool.tile([NSUM, P], f32)
            nc.vector.memset(ones_sb[:], 1.0)

            def norm_binarize(nw, n_mt, sum_emit, mm_emit, scale, bias, sink):
                """sum_emit(ps): row-sum matmuls -> ps [nw, RB];
                mm_emit(m, acc): main matmuls for m-tile; sink(m, acc, m_sb)."""
                sum_ps = spool.tile([NSUM, RB], f32, tag="sum")
                sum_emit(sum_ps[0:nw, :])
                m_row = mpool.tile([NSUM, RB], f32, tag="m_row")
                nc.scalar.activation(
                    m_row[0:nw, :], sum_ps[0:nw, :], Copy, bias=bias, scale=scale
                )
                m_ps = cpool.tile([P, RB], f32, tag="bcast")
                nc.tensor.matmul(
                    m_ps[:], ones_sb[0:nw, :], m_row[0:nw, :], start=True, stop=True
                )
                m_sb = mpool.tile([P, RB], f32, tag="m_sb")
                nc.scalar.copy(m_sb[:], m_ps[:])
                for m in range(n_mt):
                    acc = apool.tile([P, RB], f32, tag="acc")
                    mm_emit(m, acc)
                    sink(m, acc, m_sb)

            def emit_plain(rhs_list, cols):
                """rhs_list: [(tile, k, ksz, w_sb)]; cols: (c0, width)."""

                def emit(ps):
                    nmm = len(rhs_list)
                    for i, (t, k, ksz, w) in enumerate(rhs_list):
                        nc.tensor.matmul(
                            ps,
                            w[0:ksz, k, cols[0] : cols[0] + cols[1]],
                            t[0:ksz, k, :],
                            start=(i == 0),
                            stop=(i == nmm - 1),
                        )

                return emit

            def emit_dr(b_tile, w_sb, n_kt, cols):
                """DoubleRow fp8: pairs of k-tiles contracted per matmul."""

                def emit(ps):
                    npair = n_kt // 2
                    for i in range(npair):
                        nc.tensor.matmul(
                            ps,
                            w_sb[:, 2 * i : 2 * i + 2, cols[0] : cols[0] + cols[1]],
                            b_tile[:, 2 * i : 2 * i + 2, :],
                            start=(i == 0),
                            stop=(i == npair - 1),
                            perf_mode=DR,
                        )

                return emit

            for blk in range(n_blocks):
                c0 = blk * RB
                if blk not in x_tiles:
                    load_x(blk)
                xt = x_tiles.pop(blk)
                if blk + 1 < n_blocks:
                    load_x(blk + 1)  # prefetch next block's x

                rhs1 = [(xt, k, ksz, w1_sb) for k, (k0, ksz) in enumerate(kt1)]

                b1 = bpool.tile([P, len(kt2), RB], f8, tag="b1")

                def sink1(m, acc, m_sb):
                    nc.vector.tensor_tensor(b1[:, m, :], acc[:], m_sb[:], is_gt)

                norm_binarize(
                    1,
                    F1 // P,
                    emit_plain(rhs1, (F1, 1)),
                    lambda m, acc: emit_plain(rhs1, (m * P, P))(acc[:]),
                    1.0 / F1,
                    c1_over_f1,
                    sink1,
                )

                b2 = bpool.tile([P, len(kt3), RB], f8, tag="b2")

                def sink2(m, acc, m_sb):
                    nc.vector.tensor_tensor(b2[:, m, :], acc[:], m_sb[:], is_gt)

                norm_binarize(
                    NSUM,
                    F2 // P,
                    emit_dr(b1, w2_sb, len(kt2), (F2, NSUM)),
                    lambda m, acc: emit_dr(b1, w2_sb, len(kt2), (m * P, P))(acc[:]),
                    1.0 / F2,
                    0.0,
                    sink2,
                )

                b3 = bpool.tile([P, len(kt4), RB], f8, tag="b3")

                def sink3(m, acc, m_sb):
                    nc.vector.tensor_tensor(b3[:, m, :], acc[:], m_sb[:], is_gt)

                norm_binarize(
                    NSUM,
                    F3 // P,
                    emit_dr(b2, w3_sb, len(kt3), (F3, NSUM)),
                    lambda m, acc: emit_dr(b2, w3_sb, len(kt3), (m * P, P))(acc[:]),
                    1.0 / F3,
                    0.0,
                    sink3,
                )

                # ---- layer 4: plain DoubleRow matmul, no LN -------------
                acc4 = apool.tile([NC_OUT, RB], f32, tag="acc")
                emit_dr(b3, w4_sb, len(kt4), (0, NC_OUT))(acc4[:])
                out_sb = opool.tile([NC_OUT, RB], f32, tag="out")
                nc.scalar.copy(out_sb[:], acc4[:])
                nc.sync.dma_start(out=out_d[:, c0 : c0 + RB], in_=out_sb[:])

    nc.compile()
    return nc


def prep_host(x, w1, w2, w3, w4):
    """Returns (per-input dict of full arrays, C1/F1 scalar)."""
    w1b = (w1 > 0).astype(np.float32)
    top, bot = w1b[:D_IN], w1b[D_IN:]
    W1eff = top - bot
    c1 = bot.sum(0)
    W1rows = W1eff.sum(1)
    C1 = float(c1.sum())
    assert np.abs(W1rows).max() <= 256 and c1.max() <= 256
    w1m = np.zeros((K1, F1 + 1), np.float32)
    w1m[:D_IN, :F1] = W1eff
    w1m[D_IN, :F1] = c1
    w1m[:D_IN, F1] = W1rows

    def aug8(w, width):
        """fp8 layout: [binary cols | 3-way split of row-sums | zero pad]."""
        wb = (w > 0).astype(np.float32)
        nf = wb.shape[1]
        rows = wb.sum(1)
        assert rows.max() <= 3 * 16, rows.max()
        out = np.zeros((wb.shape[0], width), np.float32)
        out[:, :nf] = wb
        rem = rows
        for i in range(NSUM):
            c = np.minimum(rem, 16.0)
            out[:, nf + i] = c
            rem = rem - c
        return out.astype(fp8)

    w2m, w3m = aug8(w2, W2PAD), aug8(w3, W3PAD)
    w4m = np.zeros((F3, W4PAD), np.float32)
    w4m[:, :NC_OUT] = (w4 > 0).astype(np.float32)
    w4m = w4m.astype(fp8)

    xT = np.ascontiguousarray(x.T).astype(np.float32)  # [784, B]
    hi = xT.astype(fp16)
    r1 = xT - hi.astype(np.float32)
    lo = (r1 * LO_SCALE).astype(fp16)  # scaled chunk stays fp16-normal
    B = x.shape[0]
    # single K-stacked operand: [hi; ones; lo; zero-pad] vs [w1m; w1m/LO_SCALE; 0]
    xc = np.concatenate(
        [hi, np.ones((1, B), fp16), lo, np.zeros((KP - KC, B), fp16)], 0
    )  # [KP, B]
    w1c = np.concatenate(
        [
            w1m.astype(fp16),
            (w1m[:D_IN] / LO_SCALE).astype(fp16),
            np.zeros((KP - KC, F1 + 1), fp16),
        ],
        0,
    )  # [KP, 2049]

    arrs = {
        "xc": xc,
        "w1c": w1c,
        "w2m": w2m,
        "w3m": w3m,
        "w4m": w4m,
    }
    return arrs, C1 / F1


def _fallback_numpy(x, w1, w2, w3, w4, ln1_scale, ln1_bias, ln2_scale, ln2_bias,
                    ln3_scale, ln3_bias):
    """General path (arbitrary LN scale/bias): full fp32 LN on host."""
    h = np.concatenate([x, 1.0 - x], 1).astype(np.float32)
    for w, s, b in ((w1, ln1_scale, ln1_bias), (w2, ln2_scale, ln2_bias),
                    (w3, ln3_scale, ln3_bias)):
        a = h @ (w > 0).astype(np.float32)
        m = a.mean(1, dtype=np.float32, keepdims=True)
        v = np.mean((a - m) ** 2, axis=1, dtype=np.float32, keepdims=True)
        z = (a - m) / np.sqrt(v + 1e-6) * s + b
        h = (z > 0).astype(np.float32)
    return h @ (w4 > 0).astype(np.float32)


_CACHE = {}


def kernel(x, w1, w2, w3, w4, ln1_scale, ln1_bias, ln2_scale, ln2_bias,
           ln3_scale, ln3_bias, _trace=False):
    x = np.asarray(x, np.float32)
    fast = (
        np.all(np.asarray(ln1_scale) == 1) and np.all(np.asarray(ln1_bias) == 0)
        and np.all(np.asarray(ln2_scale) == 1) and np.all(np.asarray(ln2_bias) == 0)
        and np.all(np.asarray(ln3_scale) == 1) and np.all(np.asarray(ln3_bias) == 0)
    )
    if not fast or x.shape[0] % (N_CORES * RB) != 0:
        return _fallback_numpy(
            x, np.asarray(w1), np.asarray(w2), np.asarray(w3), np.asarray(w4),
            np.asarray(ln1_scale), np.asarray(ln1_bias), np.asarray(ln2_scale),
            np.asarray(ln2_bias), np.asarray(ln3_scale), np.asarray(ln3_bias),
        ).astype(np.float32)

    from concourse.bass_utils import run_bass_kernel_spmd

    arrs, c1_over_f1 = prep_host(
        x, np.asarray(w1), np.asarray(w2), np.asarray(w3), np.asarray(w4)
    )
    B = x.shape[0]
    R = B // N_CORES
    n_blocks = R // RB

    key = (n_blocks, round(c1_over_f1, 9))
    if key not in _CACHE:
        _CACHE[key] = build_bass(n_blocks, c1_over_f1)
    nc = _CACHE[key]

    in_maps = []
    for c in range(N_CORES):
        sl = slice(c * R, (c + 1) * R)
        m = {
            "xc": np.ascontiguousarray(arrs["xc"][:, sl]),
            "w1c": arrs["w1c"],
            "w2m": arrs["w2m"],
            "w3m": arrs["w3m"],
            "w4m": arrs["w4m"],
        }
        in_maps.append(m)

    res = run_bass_kernel_spmd(
        nc, in_maps, core_ids=list(range(N_CORES)), trace=_trace
    )
    out = np.concatenate([res.results[c]["out"] for c in range(N_CORES)], axis=1)
    if _trace:
        kernel._last_result = res
    return np.ascontiguousarray(out.T).astype(np.float32)


# revision 30
# speedup vs baseline: 1.8444x; 1.0324x over previous
"""Trainium2 Bass kernel for nn_BinaryNN (binary MLP forward pass).

Strategy (8-core data parallel over the batch):
  - Forward of _binarize_weight / _binary_activation is exactly (x > 0), so all
    hidden activations are 0/1 and layers 2-4 are exact integer matmuls -> bf16.
  - concat([x, 1-x]) @ W1b == x @ (W1top - W1bot) + colsum(W1bot): halves K to 784.
    x is split into 3 bf16 chunks (hi+mid+lo, 24 mantissa bits) for fp32-grade
    accuracy on the one real-valued matmul.
  - LayerNorm(scale=1, bias=0) followed by (.>0) reduces to (a > rowmean(a)).
    Row sums arrive as one extra M=1 matmul column (weights augmented with their
    row-sums), broadcast to 128 partitions with a K=1 ones-matmul, and the
    binarization is a single DVE tensor_tensor(is_gt) per tile.
  - Feature-major layout [features, rows] on chip: no transposes anywhere on
    device; the host pre-transposes x and transposes the [10, B] result back.
"""

import sys

if "/opt/trn_rl_repo" not in sys.path:
    sys.path.insert(0, "/opt/trn_rl_repo")

import numpy as np
import ml_dtypes

bf16 = ml_dtypes.bfloat16
fp16 = np.float16
fp8 = ml_dtypes.float8_e4m3
LO_SCALE = 4096.0  # 2**12: keeps the low fp16 chunk of x in the normal range

# fp8 weight matrices pad their free dim so the DoubleRow "two"-step is 16B-aligned
W2PAD, W3PAD, W4PAD = 1040, 528, 16
NSUM = 3  # row-sum ints (<=48) split into 3 fp8-exact (<=16) columns

N_CORES = 8
B_FULL = 32768
P = 128
RB = 512  # rows per block (PSUM bank = 512 fp32)

D_IN = 784
K1 = 785  # 784 + constant-one row carrying colsum(W1bot)
KC = K1 + D_IN  # 1569: hi chunk (with ones row) and scaled lo chunk stacked in K
KP = 1664  # KC zero-padded to 13*128 so x/w1 move as single 3D-AP DMAs
F1, F2, F3, NC_OUT = 2048, 1024, 512, 10


def _ktiles(n):
    return [(k0, min(P, n - k0)) for k0 in range(0, n, P)]


def build_bass(n_blocks, c1_over_f1):
    import concourse.bass as bass  # noqa: F401
    import concourse.mybir as mybir
    import concourse.tile as tile
    from concourse import bacc

    f32 = mybir.dt.float32
    f16 = mybir.dt.float16
    f8 = mybir.dt.float8e4
    DR = mybir.MatmulPerfMode.DoubleRow
    Copy = mybir.ActivationFunctionType.Copy
    is_gt = mybir.AluOpType.is_gt

    R = n_blocks * RB
    nc = bacc.Bacc("TRN2", target_bir_lowering=False, debug=False, num_devices=N_CORES)

    xc_d = nc.dram_tensor("xc", [KP, R], f16, kind="ExternalInput")
    w1_d = nc.dram_tensor("w1c", [KP, F1 + 1], f16, kind="ExternalInput")
    w2_d = nc.dram_tensor("w2m", [F1, W2PAD], f8, kind="ExternalInput")
    w3_d = nc.dram_tensor("w3m", [F2, W3PAD], f8, kind="ExternalInput")
    w4_d = nc.dram_tensor("w4m", [F3, W4PAD], f8, kind="ExternalInput")
    out_d = nc.dram_tensor("out", [NC_OUT, R], f32, kind="ExternalOutput")

    kt1 = _ktiles(KP)  # 13 tiles of 128
    kt2 = _ktiles(F1)  # 16
    kt3 = _ktiles(F2)  # 8
    kt4 = _ktiles(F3)  # 4

    with tile.TileContext(nc) as tc:
        with (
            tc.tile_pool(name="wpool", bufs=1) as wpool,
            tc.tile_pool(name="xpool", bufs=2) as xpool,
            tc.tile_pool(name="bpool", bufs=2) as bpool,
            tc.tile_pool(name="mpool", bufs=3) as mpool,
            tc.tile_pool(name="opool", bufs=2) as opool,
            tc.tile_pool(name="apool", bufs=4, space="PSUM") as apool,
            tc.tile_pool(name="spool", bufs=2, space="PSUM") as spool,
            tc.tile_pool(name="cpool", bufs=2, space="PSUM") as cpool,
        ):
            # ---- persistent weights (single 3D-AP DMAs) -----------------
            # DMA transfers drain roughly in issue order: block-0 x first,
            # then w1 column-chunk 0 — the minimal set for the first m-tiles.
            xr = xc_d[:, :].rearrange("(t p) r -> p t r", p=P)
            x_tiles = {}

            def load_x(blk):
                t = xpool.tile([P, len(kt1), RB], f16, tag="xc")
                c0 = blk * RB
                nc.sync.dma_start(out=t[:], in_=xr[:, :, c0 : c0 + RB])
                x_tiles[blk] = t

            load_x(0)

            wr1 = w1_d[:, :].rearrange("(t p) j -> p t j", p=P)
            w1_sb = wpool.tile([P, len(kt1), F1 + 1], f16)
            # tiny sum-column first (block 0's row-sum matmuls need it), then
            # column-chunked so early m-tiles start before all of w1 lands
            nc.sync.dma_start(out=w1_sb[:, :, F1 : F1 + 1], in_=wr1[:, :, F1 : F1 + 1])
            for c0w in range(0, F1, 512):
                cw = min(512, F1 - c0w)
                nc.sync.dma_start(
                    out=w1_sb[:, :, c0w : c0w + cw], in_=wr1[:, :, c0w : c0w + cw]
                )

            w2_sb = wpool.tile([P, len(kt2), W2PAD], f8)
            nc.sync.dma_start(
                out=w2_sb[:], in_=w2_d[:, :].rearrange("(t p) j -> p t j", p=P)
            )
            w3_sb = wpool.tile([P, len(kt3), W3PAD], f8)
            nc.sync.dma_start(
                out=w3_sb[:], in_=w3_d[:, :].rearrange("(t p) j -> p t j", p=P)
            )
            w4_sb = wpool.tile([P, len(kt4), W4PAD], f8)
            nc.sync.dma_start(
                out=w4_sb[:], in_=w4_d[:, :].rearrange("(t p) j -> p t j", p=P)
            )
            ones_sb = wpool.tile([NSUM, P], f32)
            nc.vector.memset(ones_sb[:], 1.0)

            def norm_binarize(nw, n_mt, sum_emit, mm_emit, scale, bias, sink):
                """sum_emit(ps): row-sum matmuls -> ps [nw, RB];
                mm_emit(m, acc): main matmuls for m-tile; sink(m, acc, m_sb)."""
                sum_ps = spool.tile([NSUM, RB], f32, tag="sum")
                sum_emit(sum_ps[0:nw, :])
                m_row = mpool.tile([NSUM, RB], f32, tag="m_row")
                nc.scalar.activation(
                    m_row[0:nw, :], sum_ps[0:nw, :], Copy, bias=bias, scale=scale
                )
                m_ps = cpool.tile([P, RB], f32, tag="bcast")
                nc.tensor.matmul(
                    m_ps[:], ones_sb[0:nw, :], m_row[0:nw, :], start=True, stop=True
                )
                m_sb = mpool.tile([P, RB], f32, tag="m_sb")
                nc.scalar.copy(m_sb[:], m_ps[:])
                for m in range(n_mt):
                    acc = apool.tile([P, RB], f32, tag="acc")
                    mm_emit(m, acc)
                    sink(m, acc, m_sb)

            def emit_plain(rhs_list, cols):
                """rhs_list: [(tile, k, ksz, w_sb)]; cols: (c0, width)."""

                def emit(ps):
                    nmm = len(rhs_list)
                    for i, (t, k, ksz, w) in enumerate(rhs_list):
                        nc.tensor.matmul(
                            ps,
                            w[0:ksz, k, cols[0] : cols[0] + cols[1]],
                            t[0:ksz, k, :],
                            start=(i == 0),
                            stop=(i == nmm - 1),
                        )

                return emit

            def emit_dr(b_tile, w_sb, n_kt, cols):
                """DoubleRow fp8: pairs of k-tiles contracted per matmul."""

                def emit(ps):
                    npair = n_kt // 2
                    for i in range(npair):
                        nc.tensor.matmul(
                            ps,
                            w_sb[:, 2 * i : 2 * i + 2, cols[0] : cols[0] + cols[1]],
                            b_tile[:, 2 * i : 2 * i + 2, :],
                            start=(i == 0),
                            stop=(i == npair - 1),
                            perf_mode=DR,
                        )

                return emit

            for blk in range(n_blocks):
                c0 = blk * RB
                if blk not in x_tiles:
                    load_x(blk)
                xt = x_tiles.pop(blk)
                if blk + 1 < n_blocks:
                    load_x(blk + 1)  # prefetch next block's x

                rhs1 = [(xt, k, ksz, w1_sb) for k, (k0, ksz) in enumerate(kt1)]

                b1 = bpool.tile([P, len(kt2), RB], f8, tag="b1")

                def sink1(m, acc, m_sb):
                    nc.vector.tensor_tensor(b1[:, m, :], acc[:], m_sb[:], is_gt)

                norm_binarize(
                    1,
                    F1 // P,
                    emit_plain(rhs1, (F1, 1)),
                    lambda m, acc: emit_plain(rhs1, (m * P, P))(acc[:]),
                    1.0 / F1,
                    c1_over_f1,
                    sink1,
                )

                b2 = bpool.tile([P, len(kt3), RB], f8, tag="b2")

                def sink2(m, acc, m_sb):
                    nc.vector.tensor_tensor(b2[:, m, :], acc[:], m_sb[:], is_gt)

                norm_binarize(
                    NSUM,
                    F2 // P,
                    emit_dr(b1, w2_sb, len(kt2), (F2, NSUM)),
                    lambda m, acc: emit_dr(b1, w2_sb, len(kt2), (m * P, P))(acc[:]),
                    1.0 / F2,
                    0.0,
                    sink2,
                )

                b3 = bpool.tile([P, len(kt4), RB], f8, tag="b3")

                def sink3(m, acc, m_sb):
                    nc.vector.tensor_tensor(b3[:, m, :], acc[:], m_sb[:], is_gt)

                norm_binarize(
                    NSUM,
                    F3 // P,
                    emit_dr(b2, w3_sb, len(kt3), (F3, NSUM)),
                    lambda m, acc: emit_dr(b2, w3_sb, len(kt3), (m * P, P))(acc[:]),
                    1.0 / F3,
                    0.0,
                    sink3,
                )

                # ---- layer 4: plain DoubleRow matmul, no LN -------------
                acc4 = apool.tile([NC_OUT, RB], f32, tag="acc")
                emit_dr(b3, w4_sb, len(kt4), (0, NC_OUT))(acc4[:])
                out_sb = opool.tile([NC_OUT, RB], f32, tag="out")
                nc.scalar.copy(out_sb[:], acc4[:])
                nc.sync.dma_start(out=out_d[:, c0 : c0 + RB], in_=out_sb[:])

    nc.compile()
    return nc


def prep_host(x, w1, w2, w3, w4):
    """Returns (per-input dict of full arrays, C1/F1 scalar)."""
    w1b = (w1 > 0).astype(np.float32)
    top, bot = w1b[:D_IN], w1b[D_IN:]
    W1eff = top - bot
    c1 = bot.sum(0)
    W1rows = W1eff.sum(1)
    C1 = float(c1.sum())
    assert np.abs(W1rows).max() <= 256 and c1.max() <= 256
    w1m = np.zeros((K1, F1 + 1), np.float32)
    w1m[:D_IN, :F1] = W1eff
    w1m[D_IN, :F1] = c1
    w1m[:D_IN, F1] = W1rows

    def aug8(w, width):
        """fp8 layout: [binary cols | 3-way split of row-sums | zero pad]."""
        wb = (w > 0).astype(np.float32)
        nf = wb.shape[1]
        rows = wb.sum(1)
        assert rows.max() <= 3 * 16, rows.max()
        out = np.zeros((wb.shape[0], width), np.float32)
        out[:, :nf] = wb
        rem = rows
        for i in range(NSUM):
            c = np.minimum(rem, 16.0)
            out[:, nf + i] = c
            rem = rem - c
        return out.astype(fp8)

    w2m, w3m = aug8(w2, W2PAD), aug8(w3, W3PAD)
    w4m = np.zeros((F3, W4PAD), np.float32)
    w4m[:, :NC_OUT] = (w4 > 0).astype(np.float32)
    w4m = w4m.astype(fp8)

    xT = np.ascontiguousarray(x.T).astype(np.float32)  # [784, B]
    hi = xT.astype(fp16)
    r1 = xT - hi.astype(np.float32)
    lo = (r1 * LO_SCALE).astype(fp16)  # scaled chunk stays fp16-normal
    B = x.shape[0]
    # single K-stacked operand: [hi; ones; lo; zero-pad] vs [w1m; w1m/LO_SCALE; 0]
    xc = np.concatenate(
        [hi, np.ones((1, B), fp16), lo, np.zeros((KP - KC, B), fp16)], 0
    )  # [KP, B]
    w1c = np.concatenate(
        [
            w1m.astype(fp16),
            (w1m[:D_IN] / LO_SCALE).astype(fp16),
            np.zeros((KP - KC, F1 + 1), fp16),
        ],
        0,
    )  # [KP, 2049]

    arrs = {
        "xc": xc,
        "w1c": w1c,
        "w2m": w2m,
        "w3m": w3m,
        "w4m": w4m,
    }
    return arrs, C1 / F1


def _fallback_numpy(x, w1, w2, w3, w4, ln1_scale, ln1_bias, ln2_scale, ln2_bias,
                    ln3_scale, ln3_bias):
    """General path (arbitrary LN scale/bias): full fp32 LN on host."""
    h = np.concatenate([x, 1.0 - x], 1).astype(np.float32)
    for w, s, b in ((w1, ln1_scale, ln1_bias), (w2, ln2_scale, ln2_bias),
                    (w3, ln3_scale, ln3_bias)):
        a = h @ (w > 0).astype(np.float32)
        m = a.mean(1, dtype=np.float32, keepdims=True)
        v = np.mean((a - m) ** 2, axis=1, dtype=np.float32, keepdims=True)
        z = (a - m) / np.sqrt(v + 1e-6) * s + b
        h = (z > 0).astype(np.float32)
    return h @ (w4 > 0).astype(np.float32)


_CACHE = {}


def kernel(x, w1, w2, w3, w4, ln1_scale, ln1_bias, ln2_scale, ln2_bias,
           ln3_scale, ln3_bias, _trace=False):
    x = np.asarray(x, np.float32)
    fast = (
        np.all(np.asarray(ln1_scale) == 1) and np.all(np.asarray(ln1_bias) == 0)
        and np.all(np.asarray(ln2_scale) == 1) and np.all(np.asarray(ln2_bias) == 0)
        and np.all(np.asarray(ln3_scale) == 1) and np.all(np.asarray(ln3_bias) == 0)
    )
    if not fast or x.shape[0] % (N_CORES * RB) != 0:
        return _fallback_numpy(
            x, np.asarray(w1), np.asarray(w2), np.asarray(w3), np.asarray(w4),
            np.asarray(ln1_scale), np.asarray(ln1_bias), np.asarray(ln2_scale),
            np.asarray(ln2_bias), np.asarray(ln3_scale), np.asarray(ln3_bias),
        ).astype(np.float32)

    from concourse.bass_utils import run_bass_kernel_spmd

    arrs, c1_over_f1 = prep_host(
        x, np.asarray(w1), np.asarray(w2), np.asarray(w3), np.asarray(w4)
    )
    B = x.shape[0]
    R = B // N_CORES
    n_blocks = R // RB

    key = (n_blocks, round(c1_over_f1, 9))
    if key not in _CACHE:
        _CACHE[key] = build_bass(n_blocks, c1_over_f1)
    nc = _CACHE[key]

    in_maps = []
    for c in range(N_CORES):
        sl = slice(c * R, (c + 1) * R)
        m = {
            "xc": np.ascontiguousarray(arrs["xc"][:, sl]),
            "w1c": arrs["w1c"],
            "w2m": arrs["w2m"],
            "w3m": arrs["w3m"],
            "w4m": arrs["w4m"],
        }
        in_maps.append(m)

    res = run_bass_kernel_spmd(
        nc, in_maps, core_ids=list(range(N_CORES)), trace=_trace
    )
    out = np.concatenate([res.results[c]["out"] for c in range(N_CORES)], axis=1)
    if _trace:
        kernel._last_result = res
    return np.ascontiguousarray(out.T).astype(np.float32)
